# revision 21
# baseline (speedup 1.0000x reference)
"""GCN layer kernel for Trainium2 (Bass/Tile), data-parallel over batch.

Per core (one batch element):
    out = relu(D^-1/2 A D^-1/2 (X W^T + b))

Decomposition: with d = deg^-1/2,
    out^T[o, r] = relu( sum_c ATd[c, r] * y[c, o]  +  b[o] * v[r] )
where ATd = (D^-1/2 A)^T, y = (D^-1/2 X) W^T, v[r] = d_r * (A @ d)[r]: both
diagonal scales are folded into the operands and the bias becomes a rank-1
term entering PSUM as 1-partition matmuls, so the drain is a pure relu.

Host-side prep per core (numpy: layout/dtype marshaling + O(N^2) reductions):
ATd bf16 (transposed: the tensor engine contracts over partitions; bf16 halves
the HBM traffic that bottlenecked the baseline), [W^T | first half of Xd^T]
packed [128, 2560] so the first mm1 block gates on a single DMA-completion
receipt (~2us each - measured), second Xd^T half [128, 2048], bias|v packed
[1, 2304]. deg/d on host: deg needs full A rows, which live across all 16
device tiles of ATd; computing it on device would serialize loads against
compute. Output returns transposed [256, 2048] bf16; host casts + transposes.

Device schedule (per core) — the PE stream is the critical path (the 256
N=256 products are at the bf16 roofline), so everything is shaped to stream
at the 109 ns/matmul issue floor with zero in-order-queue stalls:
  - HWDGE loads: bv on the ACT ring in parallel with [wx, xt, 16 x 512 KB ATd
    tiles] on the SP ring (each dma_start costs ~0.6us of serial issue time
    on its sequencer - measured).
  - mm1 (y = Xd W^T): 16 two-matmul accumulation chains interleaved 8-way
    across all 8 PSUM banks so the same-region read-modify-write RAW never
    stalls the PE; drains alternate ACT/DVE. Doubles as HAM warmup.
  - rank-1 bias init: 16 one-partition matmuls b_chunk x v_slice open the 8
    recycled product banks (start=True once per bank clears has_written; the
    second half-region's first write lands on cleared bits and overwrites).
  - main matmul: out^T accumulates per arriving ATd tile k: 16 products of
    N=256 (512-col moving disables the LDWEIGHTS pull-ahead and runs ~4.5x
    slower - measured), stationary = y chunk, moving = ATd slice.
  - last round interleaves per-bank pure-relu drains (alternating ACT / DVE)
    and 4 x 256 KB output DMAs split across the SP and ACT HWDGE rings.
"""

from contextlib import ExitStack

import ml_dtypes
import numpy as np

import concourse.bacc as bacc
import concourse.mybir as mybir
import concourse.tile as tile
from concourse.bass_utils import run_bass_kernel_spmd

B = 8
N = 2048
F = 256
P = 128
NT = N // P  # 16 A^T row tiles
FT = F // P  # 2 feature tiles
RQ = 4  # r-quarters (one PSUM bank each)
RW = N // RQ  # 512
HN = N // 2
F32 = mybir.dt.float32
BF16 = mybir.dt.bfloat16
COPY = mybir.ActivationFunctionType.Copy
RELU = mybir.ActivationFunctionType.Relu
ADD = mybir.AluOpType.add
MAX = mybir.AluOpType.max
BF = ml_dtypes.bfloat16

HOST_MM1 = False  # ship y precomputed on host instead of mm1 on device


def _emit(ctx: ExitStack, tc: tile.TileContext, nc, AT, XT, WT, BV, OUT):
    const = ctx.enter_context(tc.tile_pool(name="const", bufs=1))
    atp = ctx.enter_context(tc.tile_pool(name="atp", bufs=1))
    psum = ctx.enter_context(tc.tile_pool(name="psum", bufs=8, space="PSUM"))

    bv_sb = const.tile([1, F + N], BF16, tag="bv")
    y_big = const.tile([P, NT * F], BF16, tag="y")
    out_t = const.tile([P, FT * N], BF16, tag="out")
    at_big = atp.tile([P, NT * N], BF16, tag="at")
    bias_ap = bv_sb[:, :F]
    v_ap = bv_sb[:, F:]

    # input DMAs (HWDGE SP ring): [wt | xt-half0] packed so the first mm1
    # block gates on a single DMA receipt, then xt-half1, then ATd tiles;
    # bv issues in parallel on the ACT ring
    nc.scalar.dma_start(out=bv_sb[:, :], in_=BV[:, :])
    if HOST_MM1:
        nc.sync.dma_start(
            out=y_big[:, :].rearrange("p (k o) -> p k o", k=NT),
            in_=XT.rearrange("(k p) o -> p k o", p=P),
        )
        wx_sb = xt_sb = None
    else:
        wx_sb = const.tile([P, FT * F + N], BF16, tag="wx")
        xt_sb = const.tile([P, N], BF16, tag="xt")
        nc.sync.dma_start(out=wx_sb[:, :], in_=WT[:, :])
        nc.sync.dma_start(out=xt_sb[:, :], in_=XT[:, :])
    for k in range(NT):
        nc.sync.dma_start(
            out=at_big[:, k * N : (k + 1) * N], in_=AT[k * P : (k + 1) * P, :]
        )

    # ---- PE HAM warmup: junk matmuls on a memset tile from ~t=5.5us, so
    # the clock gate is at 8/8 before mm1's operands arrive (the first
    # ~3.4us of real matmuls otherwise run at 1.2 GHz - measured) ----
    ones1 = const.tile([1, P], BF16, tag="ones")
    nc.vector.memset(ones1[:, :], 1.0)
    warm = [
        psum.tile([P, 2 * F], F32, tag="bank", name=f"warm_{i}") for i in range(2)
    ]
    for j in range(32):
        nc.tensor.matmul(
            warm[(j // 2) % 2][:1, (j % 2) * F : (j % 2) * F + P],
            ones1[:, :1],
            ones1[:, :],
            start=True,
            stop=True,
            skip_group_check=True,
        )

    # ---- mm1: y = Xd @ W^T, 16 2-chains interleaved 8-way over all banks ----
    if not HOST_MM1:
        mm1t = [
            psum.tile([P, 2 * F], F32, tag="bank", name=f"mm1_{i}") for i in range(8)
        ]
        wt_sb = wx_sb[:, : FT * F]

        def reg_of(k):
            return mm1t[k % 8][:, (k // 8) * F : (k // 8 + 1) * F]

        def xt_chunk(k, phi):
            # [phi(1024)] blocks; half 0 packed after wt in wx, half 1 in xt
            tile_ap, base = (wx_sb, FT * F) if k < 8 else (xt_sb, 0)
            base += phi * HN
            return tile_ap[:, base + (k % 8) * P : base + (k % 8 + 1) * P]

        for blk in range(2):
            ks = range(8 * blk, 8 * blk + 8)
            for phi in range(FT):
                for k in ks:
                    nc.tensor.matmul(
                        reg_of(k),
                        xt_chunk(k, phi),
                        wt_sb[:, phi * F : (phi + 1) * F],
                        start=(phi == 0),
                        stop=(phi == FT - 1),
                        skip_group_check=True,
                    )
            for k in ks:
                dst = y_big[:, k * F : (k + 1) * F]
                if k % 2 == 0:
                    nc.scalar.activation(dst, reg_of(k), COPY)
                else:
                    nc.vector.tensor_scalar(
                        out=dst, in0=reg_of(k), scalar1=0.0, scalar2=None, op0=ADD
                    )

    # ---- product banks + rank-1 bias init: psum = b ⊗ v ----
    banks = {}
    for oc in range(FT):
        for rc in range(RQ):
            banks[(oc, rc)] = psum.tile(
                [P, 2 * F], F32, tag="bank", name=f"bank_{oc}_{rc}"
            )
    for oc in range(FT):
        for rc in range(RQ):
            for hf in range(2):
                nc.tensor.matmul(
                    banks[(oc, rc)][:, hf * F : (hf + 1) * F],
                    bias_ap[:, oc * P : (oc + 1) * P],
                    v_ap[:, rc * RW + hf * F : rc * RW + (hf + 1) * F],
                    start=(hf == 0),
                    stop=False,
                    skip_group_check=True,
                )

    # ---- main matmul: out^T[o, r] += sum_c y[c, o] ATd[c, r] ----
    def emit_product(k, rc, oc, hf):
        nc.tensor.matmul(
            banks[(oc, rc)][:, hf * F : (hf + 1) * F],
            y_big[:, k * F + oc * P : k * F + (oc + 1) * P],
            at_big[:, k * N + rc * RW + hf * F : k * N + rc * RW + (hf + 1) * F],
            start=False,
            stop=(k == NT - 1),
            skip_group_check=True,
        )

    for k in range(NT - 1):
        for rc in range(RQ):
            for oc in range(FT):
                for hf in range(2):
                    emit_product(k, rc, oc, hf)

    # ---- last round: interleave products, pure-relu drains, output DMAs ----
    for oc in range(FT):
        for rc in range(RQ):
            for hf in range(2):
                emit_product(NT - 1, rc, oc, hf)
            src = banks[(oc, rc)][:, : 2 * F]
            dst = out_t[:, oc * N + rc * RW : oc * N + (rc + 1) * RW]
            if rc % 2 == 0:
                nc.scalar.activation(dst, src, RELU)
            else:
                nc.vector.tensor_scalar(
                    out=dst, in0=src, scalar1=0.0, scalar2=None, op0=MAX
                )
            if rc % 2 == 1:
                half = rc // 2
                eng = nc.sync if half == 0 else nc.scalar
                eng.dma_start(
                    out=OUT[oc * P : (oc + 1) * P, half * 2 * RW : (half + 1) * 2 * RW],
                    in_=out_t[
                        :, oc * N + half * 2 * RW : oc * N + (half + 1) * 2 * RW
                    ],
                )


_cached_nc = None


def _build():
    nc = bacc.Bacc("TRN2", target_bir_lowering=False, debug=False)
    AT = nc.dram_tensor("at", [N, N], BF16, kind="ExternalInput").ap()
    xt_shape = [N, F] if HOST_MM1 else [P, N]
    XT = nc.dram_tensor("xt", xt_shape, BF16, kind="ExternalInput").ap()
    wt_shape = [P, FT * F] if HOST_MM1 else [P, FT * F + N]
    WT = nc.dram_tensor("wt", wt_shape, BF16, kind="ExternalInput").ap()
    BV = nc.dram_tensor("bv", [1, F + N], BF16, kind="ExternalInput").ap()
    OUT = nc.dram_tensor("out", [F, N], BF16, kind="ExternalOutput").ap()
    with tile.TileContext(nc) as tc:
        with ExitStack() as ctx:
            _emit(ctx, tc, nc, AT, XT, WT, BV, OUT)
    nc.compile()
    return nc


def get_nc():
    global _cached_nc
    if _cached_nc is None:
        _cached_nc = _build()
    return _cached_nc


def make_in_maps(node_features, adj_matrix, W, b):
    node_features = np.asarray(node_features, dtype=np.float32)
    adj_matrix = np.asarray(adj_matrix, dtype=np.float32)
    W = np.asarray(W, dtype=np.float32)
    b32 = np.asarray(b, dtype=np.float32)
    wt_bf = W.T.astype(BF)  # [f, o]
    # packed [128, 512]: wt[p, phi*F + o] = W.T[phi*128 + p, o]
    wt = np.ascontiguousarray(
        np.concatenate([wt_bf[phi * P : (phi + 1) * P, :] for phi in range(FT)], axis=1)
    )
    maps = []
    for c in range(B):
        adj = adj_matrix[c]
        deg = adj.sum(axis=1, dtype=np.float32)
        with np.errstate(divide="ignore"):
            d = deg**-0.5
        d = np.where(np.isfinite(d), d, 0.0).astype(np.float32)
        xd = node_features[c] * d[:, None]  # D^-1/2 X
        if HOST_MM1:
            xt = np.ascontiguousarray((xd @ W.T).astype(BF))  # y rows
            wt_c = wt
        else:
            xdt = xd.T.astype(BF)  # [f, m]
            # wx = [wt | xt-half0 phi blocks]; xt = [xt-half1 phi blocks]
            wt_c = np.ascontiguousarray(
                np.concatenate(
                    [wt]
                    + [xdt[phi * P : (phi + 1) * P, :HN] for phi in range(FT)],
                    axis=1,
                )
            )
            xt = np.ascontiguousarray(
                np.concatenate(
                    [xdt[phi * P : (phi + 1) * P, HN:] for phi in range(FT)], axis=1
                )
            )
        v = (d * (adj @ d)).astype(BF)
        bv = np.ascontiguousarray(
            np.concatenate([b32.astype(BF).reshape(1, F), v.reshape(1, N)], axis=1)
        )
        maps.append(
            {
                # (D^-1/2 A)^T: output-row scale folded in before the bf16 cast
                "at": np.ascontiguousarray((adj * d[:, None]).astype(BF).T),
                "xt": xt,
                "wt": wt_c,
                "bv": bv,
            }
        )
    return maps


def unpack_out(arr):
    """Device output [F, N] bf16 -> full-precision [N, F] f32."""
    return np.ascontiguousarray(np.asarray(arr).astype(np.float32).T)


def kernel(node_features, adj_matrix, W, b):
    nc = get_nc()
    in_maps = make_in_maps(node_features, adj_matrix, W, b)
    res = run_bass_kernel_spmd(nc, in_maps, core_ids=list(range(B)))
    return np.stack([unpack_out(r["out"]) for r in res.results], axis=0)


# revision 22
# speedup vs baseline: 1.0357x; 1.0357x over previous
"""GCN layer kernel for Trainium2 (Bass/Tile), data-parallel over batch.

Per core (one batch element):
    out = relu(D^-1/2 A D^-1/2 (X W^T + b))

Decomposition: with d = deg^-1/2,
    out^T[o, r] = relu( sum_c ATd[c, r] * y[c, o]  +  b[o] * v[r] )
where ATd = (D^-1/2 A)^T, y = (D^-1/2 X) W^T, v[r] = d_r * (A @ d)[r]: both
diagonal scales are folded into the operands and the bias becomes a rank-1
term entering PSUM as 1-partition matmuls, so the drain is a pure relu.

Host-side prep per core (numpy: layout/dtype marshaling + O(N^2) reductions):
ATd bf16 (transposed: the tensor engine contracts over partitions; bf16 halves
the HBM traffic that bottlenecked the baseline), [W^T | first half of Xd^T]
packed [128, 2560] so the first mm1 block gates on a single DMA-completion
receipt (~2us each - measured), second Xd^T half [128, 2048], bias|v packed
[1, 2304]. deg/d on host: deg needs full A rows, which live across all 16
device tiles of ATd; computing it on device would serialize loads against
compute. Output returns transposed [256, 2048] bf16; host casts + transposes.

Device schedule (per core) — the PE stream is the critical path (the 256
N=256 products are at the bf16 roofline), so everything is shaped to stream
at the 109 ns/matmul issue floor with zero in-order-queue stalls:
  - HWDGE loads: bv on the ACT ring in parallel with [wx, xt, 16 x 512 KB ATd
    tiles] on the SP ring (each dma_start costs ~0.6us of serial issue time
    on its sequencer - measured).
  - mm1 (y = Xd W^T): 16 two-matmul accumulation chains interleaved 8-way
    across all 8 PSUM banks so the same-region read-modify-write RAW never
    stalls the PE; drains alternate ACT/DVE. Doubles as HAM warmup.
  - rank-1 bias init: 16 one-partition matmuls b_chunk x v_slice open the 8
    recycled product banks (start=True once per bank clears has_written; the
    second half-region's first write lands on cleared bits and overwrites).
  - main matmul: out^T accumulates per arriving ATd tile k: 16 products of
    N=256 (512-col moving disables the LDWEIGHTS pull-ahead and runs ~4.5x
    slower - measured), stationary = y chunk, moving = ATd slice.
  - last round interleaves per-bank pure-relu drains (alternating ACT / DVE)
    and 4 x 256 KB output DMAs split across the SP and ACT HWDGE rings.
"""

from contextlib import ExitStack

import ml_dtypes
import numpy as np

import concourse.bacc as bacc
import concourse.mybir as mybir
import concourse.tile as tile
from concourse.bass_utils import run_bass_kernel_spmd

B = 8
N = 2048
F = 256
P = 128
NT = N // P  # 16 A^T row tiles
FT = F // P  # 2 feature tiles
RQ = 4  # r-quarters (one PSUM bank each)
RW = N // RQ  # 512
HN = N // 2
F32 = mybir.dt.float32
BF16 = mybir.dt.bfloat16
COPY = mybir.ActivationFunctionType.Copy
RELU = mybir.ActivationFunctionType.Relu
ADD = mybir.AluOpType.add
MAX = mybir.AluOpType.max
BF = ml_dtypes.bfloat16

HOST_MM1 = False  # ship y precomputed on host instead of mm1 on device


def _emit(ctx: ExitStack, tc: tile.TileContext, nc, AT, XT, WT, BV, OUT):
    const = ctx.enter_context(tc.tile_pool(name="const", bufs=1))
    atp = ctx.enter_context(tc.tile_pool(name="atp", bufs=1))
    psum = ctx.enter_context(tc.tile_pool(name="psum", bufs=8, space="PSUM"))

    bv_sb = const.tile([1, F + N], BF16, tag="bv")
    y_big = const.tile([P, NT * F], BF16, tag="y")
    out_t = const.tile([P, FT * N], BF16, tag="out")
    at_big = atp.tile([P, NT * N], BF16, tag="at")
    bias_ap = bv_sb[:, :F]
    v_ap = bv_sb[:, F:]

    # input DMAs (HWDGE SP ring): [wt | xt-half0] packed so the first mm1
    # block gates on a single DMA receipt, then xt-half1, then ATd tiles;
    # bv issues in parallel on the ACT ring
    nc.scalar.dma_start(out=bv_sb[:, :], in_=BV[:, :])
    if HOST_MM1:
        nc.sync.dma_start(
            out=y_big[:, :].rearrange("p (k o) -> p k o", k=NT),
            in_=XT.rearrange("(k p) o -> p k o", p=P),
        )
        wx_sb = xt_sb = None
    else:
        wx_sb = const.tile([P, FT * F + N], BF16, tag="wx")
        xt_sb = const.tile([P, N], BF16, tag="xt")
        nc.sync.dma_start(out=wx_sb[:, :], in_=WT[:, :])
        nc.sync.dma_start(out=xt_sb[:, :], in_=XT[:, :])
    for k in range(NT):
        nc.sync.dma_start(
            out=at_big[:, k * N : (k + 1) * N], in_=AT[k * P : (k + 1) * P, :]
        )

    # ---- PE HAM warmup: junk matmuls on a memset tile from ~t=5.5us, so
    # the clock gate is at 8/8 before mm1's operands arrive (the first
    # ~3.4us of real matmuls otherwise run at 1.2 GHz - measured) ----
    ones1 = const.tile([1, RW], BF16, tag="ones")
    nc.gpsimd.memset(ones1[:, :], 1.0)
    warm = [
        psum.tile([P, 2 * F], F32, tag="bank", name=f"warm_{i}") for i in range(2)
    ]
    for j in range(14):
        nc.tensor.matmul(
            warm[j % 2][:1, :RW],
            ones1[:, :1],
            ones1[:, :],
            start=True,
            stop=True,
            skip_group_check=True,
        )

    # ---- mm1: y = Xd @ W^T, 16 2-chains interleaved 8-way over all banks ----
    if not HOST_MM1:
        mm1t = [
            psum.tile([P, 2 * F], F32, tag="bank", name=f"mm1_{i}") for i in range(8)
        ]
        wt_sb = wx_sb[:, : FT * F]

        def reg_of(k):
            return mm1t[k % 8][:, (k // 8) * F : (k // 8 + 1) * F]

        def xt_chunk(k, phi):
            # [phi(1024)] blocks; half 0 packed after wt in wx, half 1 in xt
            tile_ap, base = (wx_sb, FT * F) if k < 8 else (xt_sb, 0)
            base += phi * HN
            return tile_ap[:, base + (k % 8) * P : base + (k % 8 + 1) * P]

        for blk in range(2):
            ks = range(8 * blk, 8 * blk + 8)
            for phi in range(FT):
                for k in ks:
                    nc.tensor.matmul(
                        reg_of(k),
                        xt_chunk(k, phi),
                        wt_sb[:, phi * F : (phi + 1) * F],
                        start=(phi == 0),
                        stop=(phi == FT - 1),
                        skip_group_check=True,
                    )
            for k in ks:
                dst = y_big[:, k * F : (k + 1) * F]
                if k % 2 == 0:
                    nc.scalar.activation(dst, reg_of(k), COPY)
                else:
                    nc.vector.tensor_scalar(
                        out=dst, in0=reg_of(k), scalar1=0.0, scalar2=None, op0=ADD
                    )

    # ---- product banks + rank-1 bias init: psum = b ⊗ v ----
    banks = {}
    for oc in range(FT):
        for rc in range(RQ):
            banks[(oc, rc)] = psum.tile(
                [P, 2 * F], F32, tag="bank", name=f"bank_{oc}_{rc}"
            )
    for oc in range(FT):
        for rc in range(RQ):
            for hf in range(2):
                nc.tensor.matmul(
                    banks[(oc, rc)][:, hf * F : (hf + 1) * F],
                    bias_ap[:, oc * P : (oc + 1) * P],
                    v_ap[:, rc * RW + hf * F : rc * RW + (hf + 1) * F],
                    start=(hf == 0),
                    stop=False,
                    skip_group_check=True,
                )

    # ---- main matmul: out^T[o, r] += sum_c y[c, o] ATd[c, r] ----
    def emit_product(k, rc, oc, hf):
        nc.tensor.matmul(
            banks[(oc, rc)][:, hf * F : (hf + 1) * F],
            y_big[:, k * F + oc * P : k * F + (oc + 1) * P],
            at_big[:, k * N + rc * RW + hf * F : k * N + rc * RW + (hf + 1) * F],
            start=False,
            stop=(k == NT - 1),
            skip_group_check=True,
        )

    for k in range(NT - 1):
        for rc in range(RQ):
            for oc in range(FT):
                for hf in range(2):
                    emit_product(k, rc, oc, hf)

    # ---- last round: interleave products, pure-relu drains, output DMAs ----
    for oc in range(FT):
        for rc in range(RQ):
            for hf in range(2):
                emit_product(NT - 1, rc, oc, hf)
            src = banks[(oc, rc)][:, : 2 * F]
            dst = out_t[:, oc * N + rc * RW : oc * N + (rc + 1) * RW]
            if rc % 2 == 0:
                nc.scalar.activation(dst, src, RELU)
            else:
                nc.vector.tensor_scalar(
                    out=dst, in0=src, scalar1=0.0, scalar2=None, op0=MAX
                )
            if rc % 2 == 1:
                half = rc // 2
                eng = nc.sync if half == 0 else nc.scalar
                eng.dma_start(
                    out=OUT[oc * P : (oc + 1) * P, half * 2 * RW : (half + 1) * 2 * RW],
                    in_=out_t[
                        :, oc * N + half * 2 * RW : oc * N + (half + 1) * 2 * RW
                    ],
                )


_cached_nc = None


def _build():
    nc = bacc.Bacc("TRN2", target_bir_lowering=False, debug=False)
    AT = nc.dram_tensor("at", [N, N], BF16, kind="ExternalInput").ap()
    xt_shape = [N, F] if HOST_MM1 else [P, N]
    XT = nc.dram_tensor("xt", xt_shape, BF16, kind="ExternalInput").ap()
    wt_shape = [P, FT * F] if HOST_MM1 else [P, FT * F + N]
    WT = nc.dram_tensor("wt", wt_shape, BF16, kind="ExternalInput").ap()
    BV = nc.dram_tensor("bv", [1, F + N], BF16, kind="ExternalInput").ap()
    OUT = nc.dram_tensor("out", [F, N], BF16, kind="ExternalOutput").ap()
    with tile.TileContext(nc) as tc:
        with ExitStack() as ctx:
            _emit(ctx, tc, nc, AT, XT, WT, BV, OUT)
    nc.compile()
    return nc


def get_nc():
    global _cached_nc
    if _cached_nc is None:
        _cached_nc = _build()
    return _cached_nc


def make_in_maps(node_features, adj_matrix, W, b):
    node_features = np.asarray(node_features, dtype=np.float32)
    adj_matrix = np.asarray(adj_matrix, dtype=np.float32)
    W = np.asarray(W, dtype=np.float32)
    b32 = np.asarray(b, dtype=np.float32)
    wt_bf = W.T.astype(BF)  # [f, o]
    # packed [128, 512]: wt[p, phi*F + o] = W.T[phi*128 + p, o]
    wt = np.ascontiguousarray(
        np.concatenate([wt_bf[phi * P : (phi + 1) * P, :] for phi in range(FT)], axis=1)
    )
    maps = []
    for c in range(B):
        adj = adj_matrix[c]
        deg = adj.sum(axis=1, dtype=np.float32)
        with np.errstate(divide="ignore"):
            d = deg**-0.5
        d = np.where(np.isfinite(d), d, 0.0).astype(np.float32)
        xd = node_features[c] * d[:, None]  # D^-1/2 X
        if HOST_MM1:
            xt = np.ascontiguousarray((xd @ W.T).astype(BF))  # y rows
            wt_c = wt
        else:
            xdt = xd.T.astype(BF)  # [f, m]
            # wx = [wt | xt-half0 phi blocks]; xt = [xt-half1 phi blocks]
            wt_c = np.ascontiguousarray(
                np.concatenate(
                    [wt]
                    + [xdt[phi * P : (phi + 1) * P, :HN] for phi in range(FT)],
                    axis=1,
                )
            )
            xt = np.ascontiguousarray(
                np.concatenate(
                    [xdt[phi * P : (phi + 1) * P, HN:] for phi in range(FT)], axis=1
                )
            )
        v = (d * (adj @ d)).astype(BF)
        bv = np.ascontiguousarray(
            np.concatenate([b32.astype(BF).reshape(1, F), v.reshape(1, N)], axis=1)
        )
        maps.append(
            {
                # (D^-1/2 A)^T: output-row scale folded in before the bf16 cast
                "at": np.ascontiguousarray((adj * d[:, None]).astype(BF).T),
                "xt": xt,
                "wt": wt_c,
                "bv": bv,
            }
        )
    return maps


def unpack_out(arr):
    """Device output [F, N] bf16 -> full-precision [N, F] f32."""
    return np.ascontiguousarray(np.asarray(arr).astype(np.float32).T)


def kernel(node_features, adj_matrix, W, b):
    nc = get_nc()
    in_maps = make_in_maps(node_features, adj_matrix, W, b)
    res = run_bass_kernel_spmd(nc, in_maps, core_ids=list(range(B)))
    return np.stack([unpack_out(r["out"]) for r in res.results], axis=0)
